# revision 1
# baseline (speedup 1.0000x reference)
"""Trainium2 Bass kernel: 2-layer LSTM decoder (nn_Decoder_3788161155715).

Reference model (see problem spec):
    h0 = x @ W_init.T + b_init          # (L=2, B, F); c0 = h0
    scan over T=512 steps of a stacked 2-layer LSTM cell fed by its own
    output; returns outs(T,B,F).reshape(B,T,F)  (raw reshape).

Strategy:
  * Data-parallel over batch: B=128 -> 8 cores x BL=16.  Weights replicated.
  * All recurrent matmuls put the *weights on the moving operand* of the PE
    (streams at 2.4 GHz warm) and the tiny per-step activations on the
    stationary operand (LDWEIGHTS cost ~P columns: 16 cols -> ~13ns).
    Gates accumulate in PSUM as (BL, 4F) with column order [I | F | O | G]
    (one 512-wide fp32 bank per region).  Biases are folded into the same
    accumulation via a K=1 matmul with a ones row.
  * Epilogue (sigmoid/tanh + cell update) runs in batch-partition layout on
    ACT/DVE; h_new is transposed back to feature-partition layout with PE
    transposes so it can be the next step's stationary operand.  The h1
    transpose of step t is emitted inside step t+1's matmul stream so it
    hides under the PE work instead of exposing the epilogue chain.
  * Loop: step 0 peeled (distinct 'out' vs 'h1' initial values), then
    For_i over the remaining 511 steps unrolled x7 (511 = 7 * 73).
"""

import numpy as np

import concourse.bacc as bacc
import concourse.tile as tile
from concourse import mybir
from concourse.bass import ds
from concourse.bass_utils import run_bass_kernel_spmd

L, B, T, D, F = 2, 128, 512, 64, 512
NCORES = 8
BL = B // NCORES            # 16 batch rows per core
G = 4 * F                   # 2048 gate columns, [I | F | O | G]
KC = F // 128               # 4 contraction chunks of 128
F32 = mybir.dt.float32
AF = mybir.ActivationFunctionType

# dtype used for the recurrent matmuls (both operands).  float32r is the
# full-rate fp32 PE mode (1 cycle/row at N>=256 vs 4 for exact float32).
MM_DT = mybir.dt.float32r




def _reorder_gates(w):
    """Reorder torch gate order [i,f,g,o] (rows) -> [i,f,o,g]."""
    return np.concatenate([w[0:F], w[F:2 * F], w[3 * F:4 * F], w[2 * F:3 * F]], axis=0)


def build(t_total=T, unroll=73, repeats=1):
    """Build the bass program. t_total = 1 (peeled) + unroll * n_iters.

    repeats>1 re-runs the whole computation (init included) that many
    times back-to-back; used only for wall-clock HW timing (the output is
    identical every repeat)."""
    assert (t_total - 1) % unroll == 0
    nc = bacc.Bacc("TRN2", target_bir_lowering=False, debug=False,
                   num_devices=NCORES)

    wmov = nc.dram_tensor("wmov", [4, KC, 128, G], MM_DT, kind="ExternalInput")
    biasm = nc.dram_tensor("biasm", [2, G], MM_DT, kind="ExternalInput")
    winit = nc.dram_tensor("winit", [D, F], MM_DT, kind="ExternalInput")
    binit = nc.dram_tensor("binit", [1, F], MM_DT, kind="ExternalInput")
    xt = nc.dram_tensor("xt", [L, D, BL], MM_DT, kind="ExternalInput")
    lft = nc.dram_tensor("lft", [KC, 128, BL], MM_DT, kind="ExternalInput")
    ident = nc.dram_tensor("ident", [BL, BL], F32, kind="ExternalInput")
    ones = nc.dram_tensor("ones", [1, BL], MM_DT, kind="ExternalInput")
    out = nc.dram_tensor("out", [t_total * BL, F], F32, kind="ExternalOutput")

    with tile.TileContext(nc) as tc:
        with (
            tc.tile_pool(name="wp", bufs=1) as wp,
            tc.tile_pool(name="st", bufs=1) as st,
            tc.tile_pool(name="wk", bufs=2) as wk,
            tc.tile_pool(name="ps", bufs=1, space="PSUM") as ps,
        ):
            # ---- load constants / weights ----
            w_sb = {}
            for m in range(4):
                for k in range(KC):
                    wt = wp.tile([128, G], MM_DT, tag=f"w{m}{k}")
                    nc.sync.dma_start(out=wt[:], in_=wmov[m, k])
                    w_sb[m, k] = wt
            bias_sb = []
            for cell in range(2):
                bt = wp.tile([1, G], MM_DT, tag=f"bias{cell}")
                nc.sync.dma_start(out=bt[:], in_=biasm[cell:cell + 1])
                bias_sb.append(bt)
            winit_sb = wp.tile([D, F], MM_DT, tag="winit")
            nc.sync.dma_start(out=winit_sb[:], in_=winit[:])
            binit_sb = wp.tile([1, F], MM_DT, tag="binit")
            nc.sync.dma_start(out=binit_sb[:], in_=binit[:])
            xt_sb = wp.tile([D, L * BL], MM_DT, tag="xt")
            for l in range(L):
                nc.sync.dma_start(out=xt_sb[:, l * BL:(l + 1) * BL], in_=xt[l])
            lft_sb = st.tile([128, KC * BL], MM_DT, tag="lft")
            for k in range(KC):
                nc.sync.dma_start(out=lft_sb[:, k * BL:(k + 1) * BL], in_=lft[k])
            ident_sb = wp.tile([BL, BL], F32, tag="ident")
            nc.sync.dma_start(out=ident_sb[:], in_=ident[:])
            ones_sb = wp.tile([1, BL], MM_DT, tag="ones")
            nc.sync.dma_start(out=ones_sb[:], in_=ones[:])

            # ---- state ----
            hT0 = st.tile([128, KC * BL], MM_DT, tag="hT0")   # h0(t-1).T
            hT1 = st.tile([128, KC * BL], MM_DT, tag="hT1")   # h1(t-1).T (== out(t-1).T for t>=1)
            c0 = st.tile([BL, F], F32, tag="c0")
            c1 = st.tile([BL, F], F32, tag="c1")
            hbp0 = st.tile([BL, F], F32, tag="hbp0")
            hbp1 = st.tile([BL, F], F32, tag="hbp1")

            import contextlib
            rep_ctx = (tc.For_i(0, repeats, 1) if repeats > 1
                       else contextlib.nullcontext())
            with rep_ctx:
                _emit_body(nc, tc, wp, st, wk, ps, w_sb, bias_sb, winit_sb,
                           binit_sb, xt_sb, lft_sb, ident_sb, ones_sb,
                           hT0, hT1, c0, c1, hbp0, hbp1, out, t_total, unroll)

    nc.compile()
    return nc


def _emit_body(nc, tc, wp, st, wk, ps, w_sb, bias_sb, winit_sb, binit_sb,
               xt_sb, lft_sb, ident_sb, ones_sb, hT0, hT1, c0, c1,
               hbp0, hbp1, out, t_total, unroll):
    if True:
        if True:
            # ---- init: h_l = x_l @ W_init.T + b_init ; c_l = h_l ----
            for l, (c_sb, hT_sb) in enumerate([(c0, hT0), (c1, hT1)]):
                gi = ps.tile([128, F], F32, tag=f"g{l}b3")
                gi2 = ps.tile([128, F], F32, tag=f"g{l}b0")
                nc.tensor.matmul(gi[0:BL, 0:F], lhsT=(xt_sb[:, l * BL:(l + 1) * BL]),
                                 rhs=(winit_sb[:]), start=True, stop=False)
                nc.tensor.matmul(gi[0:BL, 0:F], lhsT=(ones_sb[:]), rhs=(binit_sb[:]),
                                 start=False, stop=True)
                nc.vector.tensor_copy(out=c_sb[:], in_=gi[0:BL, 0:F])
                for k in range(KC):
                    nc.tensor.transpose(gi2[:, k * BL:(k + 1) * BL],
                                        c_sb[:, k * 128:(k + 1) * 128], ident_sb[:])
                nc.vector.tensor_copy(out=hT_sb[:], in_=gi2[:, 0:KC * BL])

            # psum bank order: finish G first, then I, F, O — so each gate's
            # activation overlaps the remaining matmuls (bank-level deps).
            BO = [3, 0, 1, 2]

            def epilogue(gb, c_sb, h_sb):
                # gb: [I, F, O, G] one-bank psum tiles
                tg = wk.tile([BL, F], F32, tag="tg")
                nc.scalar.activation(out=tg[:], in_=gb[3][0:BL, :], func=AF.Tanh)
                si = wk.tile([BL, F], F32, tag="si")
                nc.scalar.activation(out=si[:], in_=gb[0][0:BL, :], func=AF.Sigmoid)
                sf = wk.tile([BL, F], F32, tag="sf")
                nc.scalar.activation(out=sf[:], in_=gb[1][0:BL, :], func=AF.Sigmoid)
                so = wk.tile([BL, F], F32, tag="so")
                nc.scalar.activation(out=so[:], in_=gb[2][0:BL, :], func=AF.Sigmoid)
                tm2 = wk.tile([BL, F], F32, tag="tm2")
                nc.vector.tensor_mul(out=tm2[:], in0=si[:], in1=tg[:])
                tm1 = wk.tile([BL, F], F32, tag="tm1")
                nc.vector.tensor_mul(out=tm1[:], in0=sf[:], in1=c_sb[:])
                nc.vector.tensor_add(out=c_sb[:], in0=tm1[:], in1=tm2[:])
                tch = wk.tile([BL, F], F32, tag="tch")
                nc.scalar.activation(out=tch[:], in_=c_sb[:], func=AF.Tanh)
                nc.vector.tensor_mul(out=h_sb[:], in0=so[:], in1=tch[:])

            def bias_mms(gb, cell):
                for b in BO:
                    nc.tensor.matmul(gb[b][0:BL, :], lhsT=(ones_sb[:]),
                                     rhs=(bias_sb[cell][:, b * F:(b + 1) * F]),
                                     start=True, stop=False)

            def mm_group(gb, stat, m, stop=False):
                """Accumulate stat.T @ W_m into the 4 gate bank tiles."""
                for b in BO:
                    for k in range(KC):
                        nc.tensor.matmul(
                            gb[b][0:BL, :],
                            lhsT=(stat[:, k * BL:(k + 1) * BL]),
                            rhs=(w_sb[m, k][:, b * F:(b + 1) * F]),
                            start=False, stop=(stop and k == KC - 1))

            def tp_h1(gb_prev):
                """Transpose hbp1 -> hT1 using the prev step's G-bank tile."""
                for k in range(KC):
                    nc.tensor.transpose(gb_prev[3][:, k * BL:(k + 1) * BL],
                                        hbp1[:, k * 128:(k + 1) * 128], ident_sb[:])
                nc.vector.tensor_copy(out=hT1[:], in_=gb_prev[3][:, 0:KC * BL])

            def emit_step(ih0_stat, hh1_stat, prev_gb1, out_dst):
                # ---- cell 0 ----
                gb0 = [ps.tile([128, F], F32, tag=f"g0b{b}", name=f"g0b{b}") for b in range(4)]
                bias_mms(gb0, 0)
                mm_group(gb0, hT0, 1)                # h0(t-1) @ W_hh0.T
                if prev_gb1 is not None:
                    tp_h1(prev_gb1)                  # finish h1(t-1) -> hT1
                mm_group(gb0, ih0_stat, 0, stop=True)   # out(t-1) @ W_ih0.T
                epilogue(gb0, c0, hbp0)
                # ---- cell 1 ----
                gb1 = [ps.tile([128, F], F32, tag=f"g1b{b}", name=f"g1b{b}") for b in range(4)]
                bias_mms(gb1, 1)
                mm_group(gb1, hh1_stat, 3)           # h1(t-1) @ W_hh1.T
                for k in range(KC):                  # h0(t) -> hT0
                    nc.tensor.transpose(gb0[3][:, k * BL:(k + 1) * BL],
                                        hbp0[:, k * 128:(k + 1) * 128], ident_sb[:])
                nc.vector.tensor_copy(out=hT0[:], in_=gb0[3][:, 0:KC * BL])
                mm_group(gb1, hT0, 2, stop=True)     # h0(t) @ W_ih1.T
                epilogue(gb1, c1, hbp1)
                nc.sync.dma_start(out=out_dst, in_=hbp1[:])
                return gb1

            # ---- step 0 (peeled: ih0 reads last_feat, hh1 reads h1-init) ----
            g1p = emit_step(lft_sb, hT1, None, out[0:BL, :])
            tp_h1(g1p)

            # ---- steps 1 .. t_total-1 ----
            if t_total > 1:
                with tc.For_i(BL, t_total * BL, BL * unroll) as iv:
                    prev = None
                    for u in range(unroll):
                        dst = out[ds(iv + u * BL, BL), :]
                        prev = emit_step(hT1, hT1, prev, dst)
                    tp_h1(prev)


def host_prep(x, last_feat, W_init, b_init, W_ih0, W_hh0, b_ih0, b_hh0,
              W_ih1, W_hh1, b_ih1, b_hh1, W_out, b_out):
    """Build the shared + per-core input maps (pure layout transforms)."""
    f32 = np.float32
    wmov = np.stack([
        np.ascontiguousarray(_reorder_gates(np.asarray(w, f32)).T)
        .reshape(KC, 128, G)
        for w in (W_ih0, W_hh0, W_ih1, W_hh1)
    ]).astype(f32)
    biasm = np.stack([
        _reorder_gates(np.asarray(b_ih0, f32) + np.asarray(b_hh0, f32)),
        _reorder_gates(np.asarray(b_ih1, f32) + np.asarray(b_hh1, f32)),
    ]).astype(f32)
    winit = np.ascontiguousarray(np.asarray(W_init, f32).T)          # (D, F)
    binit = np.asarray(b_init, f32).reshape(1, F)
    ident = np.eye(BL, dtype=f32)
    ones = np.ones((1, BL), f32)
    x = np.asarray(x, f32)
    lf = np.asarray(last_feat, f32)
    in_maps = []
    for c in range(NCORES):
        bs = slice(c * BL, (c + 1) * BL)
        xtc = np.ascontiguousarray(np.transpose(x[:, bs, :], (0, 2, 1)))  # (L,D,BL)
        lftc = np.ascontiguousarray(lf[bs].T).reshape(KC, 128, BL)
        in_maps.append({
            "wmov": wmov, "biasm": biasm, "winit": winit, "binit": binit,
            "xt": xtc, "lft": lftc, "ident": ident, "ones": ones,
        })
    return in_maps


_NC_CACHE = {}


def _get_nc(t_total=T, unroll=73):
    key = (t_total, unroll)
    if key not in _NC_CACHE:
        _NC_CACHE[key] = build(t_total, unroll)
    return _NC_CACHE[key]


def run(inputs, trace=False, **kw):
    """Run on 8 cores; returns (output, BassKernelResults)."""
    nc = _get_nc()
    in_maps = host_prep(**inputs)
    res = run_bass_kernel_spmd(nc, in_maps, list(range(NCORES)), trace=trace, **kw)
    outs = np.empty((T, B, F), np.float32)
    for c in range(NCORES):
        outs[:, c * BL:(c + 1) * BL, :] = res.results[c]["out"].reshape(T, BL, F)
    return outs.reshape(B, T, F), res


def kernel(**inputs):
    out, _ = run(inputs)
    return out



# revision 2
# speedup vs baseline: 1.1455x; 1.1455x over previous
"""Trainium2 Bass kernel: 2-layer LSTM decoder (nn_Decoder_3788161155715).

Reference model (see problem spec):
    h0 = x @ W_init.T + b_init          # (L=2, B, F); c0 = h0
    scan over T=512 steps of a stacked 2-layer LSTM cell fed by its own
    output; returns outs(T,B,F).reshape(B,T,F)  (raw reshape).

Strategy:
  * Data-parallel over batch: B=128 -> 8 cores x BL=16.  Weights replicated.
  * All recurrent matmuls put the *weights on the moving operand* of the PE
    (streams at 2.4 GHz warm) and the tiny per-step activations on the
    stationary operand (LDWEIGHTS cost ~P columns: 16 cols -> ~13ns).
    Gates accumulate in PSUM as (BL, 4F) with column order [I | F | O | G]
    (one 512-wide fp32 bank per region).  Biases are folded into the same
    accumulation via a K=1 matmul with a ones row.
  * Epilogue (sigmoid/tanh + cell update) runs in batch-partition layout on
    ACT/DVE; h_new is transposed back to feature-partition layout with PE
    transposes so it can be the next step's stationary operand.  The h1
    transpose of step t is emitted inside step t+1's matmul stream so it
    hides under the PE work instead of exposing the epilogue chain.
  * Loop: step 0 peeled (distinct 'out' vs 'h1' initial values), then
    For_i over the remaining 511 steps unrolled x7 (511 = 7 * 73).
"""

import numpy as np

import concourse.bacc as bacc
import concourse.tile as tile
from concourse import mybir
from concourse.bass import ds
from concourse.bass_utils import run_bass_kernel_spmd

L, B, T, D, F = 2, 128, 512, 64, 512
NCORES = 8
BL = B // NCORES            # 16 batch rows per core
G = 4 * F                   # 2048 gate columns, [I | F | O | G]
KC = F // 128               # 4 contraction chunks of 128
F32 = mybir.dt.float32
AF = mybir.ActivationFunctionType

# dtype used for the recurrent matmuls (both operands).  float32r is the
# full-rate fp32 PE mode (1 cycle/row at N>=256 vs 4 for exact float32).
MM_DT = mybir.dt.float32r




def _reorder_gates(w):
    """Reorder torch gate order [i,f,g,o] (rows) -> [i,f,o,g]."""
    return np.concatenate([w[0:F], w[F:2 * F], w[3 * F:4 * F], w[2 * F:3 * F]], axis=0)


def build(t_total=T, unroll=73, repeats=1):
    """Build the bass program. t_total = 1 (peeled) + unroll * n_iters.

    repeats>1 re-runs the whole computation (init included) that many
    times back-to-back; used only for wall-clock HW timing (the output is
    identical every repeat)."""
    assert (t_total - 1) % unroll == 0
    nc = bacc.Bacc("TRN2", target_bir_lowering=False, debug=False,
                   num_devices=NCORES)

    wmov = nc.dram_tensor("wmov", [4, KC, 128, G], MM_DT, kind="ExternalInput")
    biasm = nc.dram_tensor("biasm", [2, G], MM_DT, kind="ExternalInput")
    winit = nc.dram_tensor("winit", [D, F], MM_DT, kind="ExternalInput")
    binit = nc.dram_tensor("binit", [1, F], MM_DT, kind="ExternalInput")
    xt = nc.dram_tensor("xt", [L, D, BL], MM_DT, kind="ExternalInput")
    lft = nc.dram_tensor("lft", [KC, 128, BL], MM_DT, kind="ExternalInput")
    ident = nc.dram_tensor("ident", [BL, BL], F32, kind="ExternalInput")
    ones = nc.dram_tensor("ones", [1, BL], MM_DT, kind="ExternalInput")
    out = nc.dram_tensor("out", [t_total * BL, F], F32, kind="ExternalOutput")

    with tile.TileContext(nc) as tc:
        with (
            tc.tile_pool(name="wp", bufs=1) as wp,
            tc.tile_pool(name="st", bufs=1) as st,
            tc.tile_pool(name="wk", bufs=2) as wk,
            tc.tile_pool(name="ps", bufs=1, space="PSUM") as ps,
        ):
            # ---- load constants / weights ----
            w_sb = {}
            for m in range(4):
                for k in range(KC):
                    wt = wp.tile([128, G], MM_DT, tag=f"w{m}{k}")
                    nc.sync.dma_start(out=wt[:], in_=wmov[m, k])
                    w_sb[m, k] = wt
            bias_sb = []
            for cell in range(2):
                bt = wp.tile([1, G], MM_DT, tag=f"bias{cell}")
                nc.sync.dma_start(out=bt[:], in_=biasm[cell:cell + 1])
                bias_sb.append(bt)
            winit_sb = wp.tile([D, F], MM_DT, tag="winit")
            nc.sync.dma_start(out=winit_sb[:], in_=winit[:])
            binit_sb = wp.tile([1, F], MM_DT, tag="binit")
            nc.sync.dma_start(out=binit_sb[:], in_=binit[:])
            xt_sb = wp.tile([D, L * BL], MM_DT, tag="xt")
            for l in range(L):
                nc.sync.dma_start(out=xt_sb[:, l * BL:(l + 1) * BL], in_=xt[l])
            lft_sb = st.tile([128, KC * BL], MM_DT, tag="lft")
            for k in range(KC):
                nc.sync.dma_start(out=lft_sb[:, k * BL:(k + 1) * BL], in_=lft[k])
            ident_sb = wp.tile([BL, BL], F32, tag="ident")
            nc.sync.dma_start(out=ident_sb[:], in_=ident[:])
            ones_sb = wp.tile([1, BL], MM_DT, tag="ones")
            nc.sync.dma_start(out=ones_sb[:], in_=ones[:])

            # ---- state ----
            hT0 = st.tile([128, KC * BL], MM_DT, tag="hT0")   # h0(t-1).T
            hT1 = st.tile([128, KC * BL], MM_DT, tag="hT1")   # h1(t-1).T (== out(t-1).T for t>=1)
            c0 = st.tile([BL, F], F32, tag="c0")
            c1 = st.tile([BL, F], F32, tag="c1")
            hbp0 = st.tile([BL, F], F32, tag="hbp0")
            hbp1 = st.tile([BL, F], F32, tag="hbp1")

            import contextlib
            rep_ctx = (tc.For_i(0, repeats, 1) if repeats > 1
                       else contextlib.nullcontext())
            with rep_ctx:
                _emit_body(nc, tc, wp, st, wk, ps, w_sb, bias_sb, winit_sb,
                           binit_sb, xt_sb, lft_sb, ident_sb, ones_sb,
                           hT0, hT1, c0, c1, hbp0, hbp1, out, t_total, unroll)

    nc.compile()
    return nc


def _emit_body(nc, tc, wp, st, wk, ps, w_sb, bias_sb, winit_sb, binit_sb,
               xt_sb, lft_sb, ident_sb, ones_sb, hT0, hT1, c0, c1,
               hbp0, hbp1, out, t_total, unroll):
    if True:
        if True:
            # ---- init: h_l = x_l @ W_init.T + b_init ; c_l = h_l ----
            for l, (c_sb, hT_sb) in enumerate([(c0, hT0), (c1, hT1)]):
                gi = ps.tile([128, F], F32, tag=f"g{l}b3")
                gi2 = ps.tile([128, F], F32, tag=f"g{l}b0")
                nc.tensor.matmul(gi[0:BL, 0:F], lhsT=(xt_sb[:, l * BL:(l + 1) * BL]),
                                 rhs=(winit_sb[:]), start=True, stop=False)
                nc.tensor.matmul(gi[0:BL, 0:F], lhsT=(ones_sb[:]), rhs=(binit_sb[:]),
                                 start=False, stop=True)
                nc.vector.tensor_copy(out=c_sb[:], in_=gi[0:BL, 0:F])
                for k in range(KC):
                    nc.tensor.transpose(gi2[:, k * BL:(k + 1) * BL],
                                        c_sb[:, k * 128:(k + 1) * 128], ident_sb[:])
                nc.vector.tensor_copy(out=hT_sb[:], in_=gi2[:, 0:KC * BL])

            # psum bank order: finish G first, then I, F, O — so each gate's
            # activation overlaps the remaining matmuls (bank-level deps).
            BO = [3, 0, 1, 2]

            def epilogue(gb, c_sb, h_sb):
                # gb: [I, F, O, G] one-bank psum tiles
                tg = wk.tile([BL, F], F32, tag="tg")
                nc.scalar.activation(out=tg[:], in_=gb[3][0:BL, :], func=AF.Tanh)
                si = wk.tile([BL, F], F32, tag="si")
                nc.scalar.activation(out=si[:], in_=gb[0][0:BL, :], func=AF.Sigmoid)
                sf = wk.tile([BL, F], F32, tag="sf")
                nc.scalar.activation(out=sf[:], in_=gb[1][0:BL, :], func=AF.Sigmoid)
                so = wk.tile([BL, F], F32, tag="so")
                nc.scalar.activation(out=so[:], in_=gb[2][0:BL, :], func=AF.Sigmoid)
                tm2 = wk.tile([BL, F], F32, tag="tm2")
                nc.vector.tensor_mul(out=tm2[:], in0=si[:], in1=tg[:])
                tm1 = wk.tile([BL, F], F32, tag="tm1")
                nc.vector.tensor_mul(out=tm1[:], in0=sf[:], in1=c_sb[:])
                nc.vector.tensor_add(out=c_sb[:], in0=tm1[:], in1=tm2[:])
                tch = wk.tile([BL, F], F32, tag="tch")
                nc.scalar.activation(out=tch[:], in_=c_sb[:], func=AF.Tanh)
                nc.vector.tensor_mul(out=h_sb[:], in0=so[:], in1=tch[:])

            def bias_mms(gb, cell):
                for b in BO:
                    nc.tensor.matmul(gb[b][0:BL, :], lhsT=(ones_sb[:]),
                                     rhs=(bias_sb[cell][:, b * F:(b + 1) * F]),
                                     start=True, stop=False)

            def mm_group(gb, stat, m, stop=False):
                """Accumulate stat.T @ W_m into the 4 gate bank tiles.

                k-outer / bank-inner: 4 consecutive matmuls share the same
                stationary operand (one LDWEIGHTS per k-chunk instead of
                one per matmul)."""
                for k in range(KC):
                    for b in BO:
                        nc.tensor.matmul(
                            gb[b][0:BL, :],
                            lhsT=(stat[:, k * BL:(k + 1) * BL]),
                            rhs=(w_sb[m, k][:, b * F:(b + 1) * F]),
                            start=False, stop=(stop and k == KC - 1))

            def tp_h1(gb_prev):
                """Transpose hbp1 -> hT1 using the prev step's G-bank tile."""
                for k in range(KC):
                    nc.tensor.transpose(gb_prev[3][:, k * BL:(k + 1) * BL],
                                        hbp1[:, k * 128:(k + 1) * 128], ident_sb[:])
                nc.vector.tensor_copy(out=hT1[:], in_=gb_prev[3][:, 0:KC * BL])

            def emit_step(ih0_stat, hh1_stat, prev_gb1, out_dst):
                # ---- cell 0 ----
                gb0 = [ps.tile([128, F], F32, tag=f"g0b{b}", name=f"g0b{b}") for b in range(4)]
                bias_mms(gb0, 0)
                mm_group(gb0, hT0, 1)                # h0(t-1) @ W_hh0.T
                if prev_gb1 is not None:
                    tp_h1(prev_gb1)                  # finish h1(t-1) -> hT1
                mm_group(gb0, ih0_stat, 0, stop=True)   # out(t-1) @ W_ih0.T
                epilogue(gb0, c0, hbp0)
                # ---- cell 1 ----
                gb1 = [ps.tile([128, F], F32, tag=f"g1b{b}", name=f"g1b{b}") for b in range(4)]
                bias_mms(gb1, 1)
                mm_group(gb1, hh1_stat, 3)           # h1(t-1) @ W_hh1.T
                for k in range(KC):                  # h0(t) -> hT0
                    nc.tensor.transpose(gb0[3][:, k * BL:(k + 1) * BL],
                                        hbp0[:, k * 128:(k + 1) * 128], ident_sb[:])
                nc.vector.tensor_copy(out=hT0[:], in_=gb0[3][:, 0:KC * BL])
                mm_group(gb1, hT0, 2, stop=True)     # h0(t) @ W_ih1.T
                epilogue(gb1, c1, hbp1)
                nc.sync.dma_start(out=out_dst, in_=hbp1[:])
                return gb1

            # ---- step 0 (peeled: ih0 reads last_feat, hh1 reads h1-init) ----
            g1p = emit_step(lft_sb, hT1, None, out[0:BL, :])
            tp_h1(g1p)

            # ---- steps 1 .. t_total-1 ----
            if t_total > 1:
                with tc.For_i(BL, t_total * BL, BL * unroll) as iv:
                    prev = None
                    for u in range(unroll):
                        dst = out[ds(iv + u * BL, BL), :]
                        prev = emit_step(hT1, hT1, prev, dst)
                    tp_h1(prev)


def host_prep(x, last_feat, W_init, b_init, W_ih0, W_hh0, b_ih0, b_hh0,
              W_ih1, W_hh1, b_ih1, b_hh1, W_out, b_out):
    """Build the shared + per-core input maps (pure layout transforms)."""
    f32 = np.float32
    wmov = np.stack([
        np.ascontiguousarray(_reorder_gates(np.asarray(w, f32)).T)
        .reshape(KC, 128, G)
        for w in (W_ih0, W_hh0, W_ih1, W_hh1)
    ]).astype(f32)
    biasm = np.stack([
        _reorder_gates(np.asarray(b_ih0, f32) + np.asarray(b_hh0, f32)),
        _reorder_gates(np.asarray(b_ih1, f32) + np.asarray(b_hh1, f32)),
    ]).astype(f32)
    winit = np.ascontiguousarray(np.asarray(W_init, f32).T)          # (D, F)
    binit = np.asarray(b_init, f32).reshape(1, F)
    ident = np.eye(BL, dtype=f32)
    ones = np.ones((1, BL), f32)
    x = np.asarray(x, f32)
    lf = np.asarray(last_feat, f32)
    in_maps = []
    for c in range(NCORES):
        bs = slice(c * BL, (c + 1) * BL)
        xtc = np.ascontiguousarray(np.transpose(x[:, bs, :], (0, 2, 1)))  # (L,D,BL)
        lftc = np.ascontiguousarray(lf[bs].T).reshape(KC, 128, BL)
        in_maps.append({
            "wmov": wmov, "biasm": biasm, "winit": winit, "binit": binit,
            "xt": xtc, "lft": lftc, "ident": ident, "ones": ones,
        })
    return in_maps


_NC_CACHE = {}


def _get_nc(t_total=T, unroll=73):
    key = (t_total, unroll)
    if key not in _NC_CACHE:
        _NC_CACHE[key] = build(t_total, unroll)
    return _NC_CACHE[key]


def run(inputs, trace=False, **kw):
    """Run on 8 cores; returns (output, BassKernelResults)."""
    nc = _get_nc()
    in_maps = host_prep(**inputs)
    res = run_bass_kernel_spmd(nc, in_maps, list(range(NCORES)), trace=trace, **kw)
    outs = np.empty((T, B, F), np.float32)
    for c in range(NCORES):
        outs[:, c * BL:(c + 1) * BL, :] = res.results[c]["out"].reshape(T, BL, F)
    return outs.reshape(B, T, F), res


def kernel(**inputs):
    out, _ = run(inputs)
    return out

